# revision 15
# baseline (speedup 1.0000x reference)
"""GQA attention + RoPE + causal softmax + output projection on 8 TRN2 cores.

Sharding: tensor-parallel over heads. Core i owns q-heads [4i, 4i+4) and
kv-head i (GQA group size 4 aligns exactly with HQ/8=4, HK/8=1).

Per-core pipeline (everything in transposed "feature-on-partitions" layout),
with stages A (projections+RoPE) and B (attention) interleaved per 512-wide
seq block so the per-(head,qblock) AllGathers start early and finish long
before stage D consumes them:

  for sb in 0..3:
    A(sb): Q^T/K^T/V^T projections for seq block sb (lhsT = weight chunk
      [Dc,128], rhs = x^T chunk [Dc,512] -> PSUM [feat,seq]); RoPE on Q^T/K^T
      via stream_shuffle + 2 muls + add; V^T PE-transposed to V [seq,128].
    B(qb=sb): per head, causal attention over k-chunks 0..4*(qb+1):
      scores^T [sk,128 x sq,512] = K^T-chunk (stationary) x Q^T (moving);
      p = exp(scores * 1/sqrt(hd)) on ACT. Diagonal chunks are column-trimmed
      (only sq >= 128*td is computed) into persistent zero-prefix tiles, and
      masked with one [128,128] triangular strip mask (DVE).
      Softmax denominator: exp chunks are pairwise tree-summed on DVE (bf16),
      then ONE all-ones [128,128] stationary matmul on the root broadcasts
      the column sums to every partition (vs one matmul per chunk before:
      removes 160 PE matmuls).
      out^T[128,sq] += V-chunk^T @ p (PE), 2-chunk software pipeline;
      attn^T = out^T * reciprocal_approx_fast(norm) (DVE) -> DMA (vector
      queue) -> per-(h,qb) AllGather [128,512]->[1024,512] on gpsimd.
  D: out^T column shard: lhsT = wo chunk, rhs = gathered attn^T chunk for
     seq block g (depends only on gathers (h, qb=g) - all long done),
     accumulated over all 4096 contraction rows. Host transposes + concats.

PSUM tags are shared across stages (8 banks total): A accumulators
psq0-3/psk/psv, B score tiles rotate over psq0-3, B out/norm use psk/psv and
extra0, V-transpose uses its own bank, D accumulators rotate over psq0-3.

Matmul operands are bf16 (1 cycle/row on PE); accumulation is fp32 in PSUM;
softmax denominator and normalization stay fp32 after the bf16 chunk tree.
"""

import numpy as np
import ml_dtypes

import concourse.bass as bass
import concourse.mybir as mybir
import concourse.tile as tile
from concourse import bacc
from concourse.bass_utils import run_bass_kernel_spmd

# Problem dims (hardcoded per contract)
B, S, D = 1, 2048, 4096
HQ, HK, HD = 32, 8, 128
NCORES = 8
HQL = HQ // NCORES          # 4 local q heads
SB = 512                    # seq block (matmul moving free dim)
NB = S // SB                # 4 seq blocks
NC_ = D // 128              # 32 contraction chunks for D
SCALE = 1.0 / float(np.sqrt(HD))

F32 = mybir.dt.float32
BF16 = mybir.dt.bfloat16

# stream_shuffle mask: swap adjacent pairs within each 32-partition quadrant
SWAP_MASK = [(i ^ 1) for i in range(32)]


def _build_nc():
    nc = bacc.Bacc(
        "TRN2", target_bir_lowering=False, debug=False, num_devices=NCORES
    )

    io = {}
    io["xT"] = nc.dram_tensor("xT", [D, S], BF16, kind="ExternalInput")
    io["wq"] = nc.dram_tensor("wq", [D, HQL * HD], BF16, kind="ExternalInput")
    io["wk"] = nc.dram_tensor("wk", [D, HD], BF16, kind="ExternalInput")
    io["wv"] = nc.dram_tensor("wv", [D, HD], BF16, kind="ExternalInput")
    io["wo"] = nc.dram_tensor("wo", [D, HQL * HD], BF16, kind="ExternalInput")
    io["cos2"] = nc.dram_tensor("cos2", [HD, S], BF16, kind="ExternalInput")
    io["sin2"] = nc.dram_tensor("sin2", [HD, S], BF16, kind="ExternalInput")
    io["maskt"] = nc.dram_tensor("maskt", [128, 128], BF16, kind="ExternalInput")
    io["ident"] = nc.dram_tensor("ident", [128, 128], BF16, kind="ExternalInput")
    io["outT"] = nc.dram_tensor("outT", [HQL * HD, S], F32, kind="ExternalOutput")

    with tile.TileContext(nc) as tc:
        _body(tc, io)
    nc.compile()
    return nc


def _body(tc, io):
    nc = tc.nc
    from contextlib import ExitStack

    ctx = ExitStack()
    with ctx:
        consts = ctx.enter_context(tc.tile_pool(name="consts", bufs=1))
        qkv = ctx.enter_context(tc.tile_pool(name="qkv", bufs=1))
        dram = ctx.enter_context(tc.tile_pool(name="dram", bufs=1, space="DRAM"))
        wpool = ctx.enter_context(tc.tile_pool(name="wpool", bufs=1))
        xpool = ctx.enter_context(tc.tile_pool(name="xpool", bufs=12))
        rpool = ctx.enter_context(tc.tile_pool(name="rpool", bufs=3))
        ppool = ctx.enter_context(tc.tile_pool(name="ppool", bufs=6))
        accp = ctx.enter_context(tc.tile_pool(name="accp", bufs=2))
        spool = ctx.enter_context(tc.tile_pool(name="spool", bufs=2))
        apool = ctx.enter_context(tc.tile_pool(name="apool", bufs=8))
        opool = ctx.enter_context(tc.tile_pool(name="opool", bufs=4))
        ps = ctx.enter_context(tc.tile_pool(name="ps", bufs=1, space="PSUM"))

        # ---- constants ----
        cos2 = consts.tile([HD, S], BF16)
        sin2 = consts.tile([HD, S], BF16)
        ident = consts.tile([128, 128], BF16)
        maskt = consts.tile([128, 128], BF16)
        ones_mat = consts.tile([128, 128], BF16)
        nc.vector.memset(ones_mat, 1.0)

        # ---- persistent per-core tensors ----
        qt_sb = [
            qkv.tile([HD, HQL, SB], BF16, name=f"qt{sb}") for sb in range(NB)
        ]
        kt_sb = [qkv.tile([HD, SB], BF16, name=f"kt{sb}") for sb in range(NB)]
        vs_sb = [
            qkv.tile([128, SB // 128, HD], BF16, name=f"vs{sb}") for sb in range(NB)
        ]
        # persistent zero-prefix exp tiles for the 4 diagonal chunk offsets;
        # prefix [0, 128*td) is zeroed once and never written again, so the
        # trimmed exp + full-width PV/denominator accumulation stay correct.
        ptd = [qkv.tile([128, SB], BF16, name=f"ptd{td}") for td in range(4)]
        for td in range(1, 4):
            nc.vector.memset(ptd[td][:, : 128 * td], 0.0)

        # per-qblock bounce + gather buffers: one AllGather per q-block
        # covering all 4 local heads (h-major rows), issued at each B(qb)
        # end -> overlaps the remaining A/B blocks and stage D. Gathered
        # rows are core-major then head: row (i*HQL + j)*128 + p.
        attn_loc = [
            dram.tile([HQL, HD, SB], BF16, name=f"aloc{qb}") for qb in range(NB)
        ]
        attn_g = [
            dram.tile(
                [NCORES * HQL * HD, SB], BF16, name=f"ag{qb}", addr_space="Shared"
            )
            for qb in range(NB)
        ]

        # ---- weight loads: first chunks spread across queues so the PE
        # starts ASAP; bulk on gpsimd (done during A(0), before collectives)
        wq_sb = wpool.tile([128, NC_, HQL * HD], BF16)
        wk_sb = wpool.tile([128, NC_, HD], BF16)
        wv_sb = wpool.tile([128, NC_, HD], BF16)
        for t in range(HQL):
            eng = [nc.scalar, nc.scalar, nc.sync, nc.sync][t]
            eng.dma_start(
                out=wq_sb[:, 0, t * 128 : (t + 1) * 128],
                in_=io["wq"][0:128, t * 128 : (t + 1) * 128],
            )
        nc.gpsimd.dma_start(out=wk_sb[:, 0, :], in_=io["wk"][0:128, :])
        nc.gpsimd.dma_start(out=wv_sb[:, 0, :], in_=io["wv"][0:128, :])
        nc.gpsimd.dma_start(out=maskt, in_=io["maskt"][:, :])
        for c in range(1, 4):
            sl = slice(c * 128, (c + 1) * 128)
            nc.gpsimd.dma_start(out=wq_sb[:, c, :], in_=io["wq"][sl, :])
            nc.gpsimd.dma_start(out=wk_sb[:, c, :], in_=io["wk"][sl, :])
            nc.gpsimd.dma_start(out=wv_sb[:, c, :], in_=io["wv"][sl, :])
        for c4 in range(1, NC_ // 4):
            sl = slice(c4 * 4 * 128, (c4 * 4 + 4) * 128)
            nc.gpsimd.dma_start(
                out=wq_sb[:, c4 * 4 : c4 * 4 + 4, :],
                in_=io["wq"][sl, :].rearrange("(c p) n -> p c n", p=128),
            )
            nc.gpsimd.dma_start(
                out=wk_sb[:, c4 * 4 : c4 * 4 + 4, :],
                in_=io["wk"][sl, :].rearrange("(c p) n -> p c n", p=128),
            )
            nc.gpsimd.dma_start(
                out=wv_sb[:, c4 * 4 : c4 * 4 + 4, :],
                in_=io["wv"][sl, :].rearrange("(c p) n -> p c n", p=128),
            )

        wo_sb = wpool.tile([128, NC_, HQL * HD], BF16)

        # xt chunk-pair loader: [128, 2, SB] tiles (chunks 2cp, 2cp+1) on the
        # scalar queue (the sync queue is kept clear: the collectives' D2D
        # transfers run there and would stall compute-critical loads).
        xts = {}

        def emit_xt(sb, cp, eng):
            t = xpool.tile([128, 2, SB], BF16, tag="xt")
            ssl = slice(sb * SB, (sb + 1) * SB)
            sl = slice(cp * 256, (cp + 1) * 256)
            eng.dma_start(
                out=t, in_=io["xT"][sl, ssl].rearrange("(c p) n -> p c n", p=128)
            )
            xts[(sb, cp)] = t

        emit_xt(0, 0, nc.sync)  # very first pair in parallel with wq chunks

        # =============== interleaved stage A(sb) + stage B(qb=sb) ===========
        for sb in range(NB):
            _stage_a_block(nc, tc, io, sb, ps, xpool, rpool, xts, emit_xt,
                           (wq_sb, wk_sb, wv_sb),
                           cos2, sin2, ident, qt_sb, kt_sb, vs_sb)
            if sb == 0:
                # wo loads fill DMA idle time from here on (gpsimd queue,
                # after the qkv weights and before most collectives)
                for c4 in range(NC_ // 4):
                    sl = slice(c4 * 4 * 128, (c4 * 4 + 4) * 128)
                    nc.gpsimd.dma_start(
                        out=wo_sb[:, c4 * 4 : c4 * 4 + 4, :],
                        in_=io["wo"][sl, :].rearrange("(c p) n -> p c n", p=128),
                    )
            if sb + 1 < NB:
                # prefetch the first 12 chunks of the next block ahead of
                # B(qb)'s exp work in the scalar queue
                for cp in range(6):
                    emit_xt(sb + 1, cp, nc.scalar)
            _stage_b_block(nc, tc, sb, ps, ppool, accp, spool, maskt, ones_mat,
                           qt_sb, kt_sb, vs_sb, ptd, attn_loc, attn_g)

        # ================= Stage D: out = attn @ wo (column shard) =========
        for g in range(NB):
            gsl = slice(g * SB, (g + 1) * SB)
            ps_d = [
                ps.tile([128, SB], F32, name=f"psd{g}_{n}", tag=f"psq{n}")
                for n in range(HQL)
            ]
            for j in range(HQL):
                for i in range(NCORES):
                    c = i * HQL + j
                    at = apool.tile([128, SB], BF16, tag="at")
                    nc.scalar.dma_start(
                        out=at, in_=attn_g[g][c * 128 : (c + 1) * 128, :]
                    )
                    first = j == 0 and i == 0
                    last = j == HQL - 1 and i == NCORES - 1
                    for n in range(HQL):
                        nc.tensor.matmul(
                            ps_d[n],
                            lhsT=wo_sb[:, c, n * 128 : (n + 1) * 128],
                            rhs=at,
                            start=first,
                            stop=last,
                        )
            for n in range(HQL):
                # alternate ACT/DVE for the PSUM evictions: halves the
                # serialized copy tail after each g-group's last matmul
                ot = opool.tile([128, SB], F32, name=f"ot{g}_{n}", tag="ot")
                if n % 2 == 0:
                    nc.scalar.copy(ot, ps_d[n])
                else:
                    nc.vector.tensor_copy(ot, ps_d[n])
                nc.sync.dma_start(
                    out=io["outT"][n * 128 : (n + 1) * 128, gsl], in_=ot
                )


def _stage_a_block(nc, tc, io, sb, ps, xpool, rpool, xts, emit_xt, weights,
                   cos2, sin2, ident, qt_sb, kt_sb, vs_sb):
    """Projections + RoPE for seq block sb."""
    ssl = slice(sb * SB, (sb + 1) * SB)
    ps_q = [
        ps.tile([128, SB], F32, name=f"psq{t}_{sb}", tag=f"psq{t}")
        for t in range(HQL)
    ]
    ps_k = ps.tile([128, SB], F32, name=f"psk_{sb}", tag="psk")
    ps_v = ps.tile([128, SB], F32, name=f"psv_{sb}", tag="psv")
    wq_sb, wk_sb, wv_sb = weights
    for c in range(NC_):
        if c % 2 == 0:
            if (sb, c // 2) not in xts:
                emit_xt(sb, c // 2, nc.scalar)
            xt2 = xts.pop((sb, c // 2))
        xt = xt2[:, c % 2, :]
        first, last = c == 0, c == NC_ - 1
        for t in range(HQL):
            nc.tensor.matmul(
                ps_q[t],
                lhsT=wq_sb[:, c, t * 128 : (t + 1) * 128],
                rhs=xt,
                start=first,
                stop=last,
            )
        nc.tensor.matmul(
            ps_k, lhsT=wk_sb[:, c, :], rhs=xt, start=first, stop=last
        )
        nc.tensor.matmul(
            ps_v, lhsT=wv_sb[:, c, :], rhs=xt, start=first, stop=last
        )

    # const loads trail this block's xt DMAs on the scalar queue
    if sb == 0:
        nc.scalar.dma_start(out=ident, in_=io["ident"][:, :])
    nc.scalar.dma_start(out=cos2[:, ssl], in_=io["cos2"][:, ssl])
    nc.scalar.dma_start(out=sin2[:, ssl], in_=io["sin2"][:, ssl])

    # PSUM evictions (ACT copies, bf16 cast). k first for sb==0 so B(0)'s
    # first scores aren't gated on the whole rope chain; q0 first otherwise.
    qc_k = rpool.tile([128, SB], BF16, name=f"qck{sb}", tag="qck")
    qc_q = [
        rpool.tile([128, SB], BF16, name=f"qc{sb}_{t}", tag=f"qc{t}")
        for t in range(HQL)
    ]

    def rope_dve(qc, dst, idx):
        sw = rpool.tile([128, SB], BF16, name=f"sw{idx}", tag="sw")
        nc.vector.stream_shuffle(sw, qc, SWAP_MASK)
        t1 = rpool.tile([128, SB], BF16, name=f"t1{idx}", tag="t1")
        nc.vector.tensor_mul(t1, qc, cos2[:, ssl])
        t2 = rpool.tile([128, SB], BF16, name=f"t2{idx}", tag="t2")
        nc.vector.tensor_mul(t2, sw, sin2[:, ssl])
        nc.vector.tensor_add(dst, t1, t2)

    if sb == 0:
        nc.scalar.copy(qc_k, ps_k)
        rope_dve(qc_k, kt_sb[sb], f"k{sb}")
        nc.scalar.copy(qc_q[0], ps_q[0])
        rope_dve(qc_q[0], qt_sb[sb][:, 0, :], f"q{sb}_0")
    else:
        nc.scalar.copy(qc_q[0], ps_q[0])
        rope_dve(qc_q[0], qt_sb[sb][:, 0, :], f"q{sb}_0")
        nc.scalar.copy(qc_k, ps_k)
        rope_dve(qc_k, kt_sb[sb], f"k{sb}")
    for t in range(1, HQL):
        nc.scalar.copy(qc_q[t], ps_q[t])
        rope_dve(qc_q[t], qt_sb[sb][:, t, :], f"q{sb}_{t}")

    # V^T -> V via XBAR DMA transpose per 128-col chunk (SBUF->SBUF, frees
    # the PE and a PSUM bank vs the PE-transpose path). The triggers follow
    # the vts ACT copy on the same (scalar) queue.
    vts = rpool.tile([128, SB], BF16, name=f"vts{sb}", tag="vts")
    nc.scalar.copy(vts, ps_v)
    for u in range(SB // 128):
        nc.scalar.dma_start(
            out=vs_sb[sb][:, u, :],
            in_=vts[:, u * 128 : (u + 1) * 128],
            transpose=True,
        )


def _stage_b_block(nc, tc, qb, ps, ppool, accp, spool, maskt, ones_mat,
                   qt_sb, kt_sb, vs_sb, ptd, attn_loc, attn_g):
    """Causal attention for q-block qb over k-chunks 0..4*(qb+1)."""
    nkc = 4 * (qb + 1)
    for h in range(HQL):
        pso = ps.tile(
            [128, SB], F32, name=f"pso{qb}_{h}", tag=("psk" if h % 2 == 0 else "psv")
        )
        psn = ps.tile(
            [128, SB], F32, name=f"psn{qb}_{h}",
            tag=("extra0" if h % 2 == 0 else "extra1"),
        )
        pts = {}
        pend = {}  # binary-counter pairwise tree accumulation on DVE

        def feed(t, _s=[0]):
            lvl = 0
            while lvl in pend:
                prev = pend.pop(lvl)
                _s[0] += 1
                nt = accp.tile(
                    [128, SB], BF16, name=f"acc{qb}_{h}_{_s[0]}", tag=f"acc{lvl}"
                )
                nc.vector.tensor_add(nt, prev, t)
                t = nt
                lvl += 1
            pend[lvl] = t

        def pv(kc):
            nc.tensor.matmul(
                pso,
                lhsT=vs_sb[kc // 4][:, kc % 4, :],
                rhs=pts.pop(kc),
                start=kc == 0,
                stop=kc == nkc - 1,
            )

        for kc in range(nkc):
            td = kc - 4 * qb
            pss = ps.tile(
                [128, SB], F32, name=f"pss{qb}_{h}_{kc}", tag=f"psq{kc % 4}"
            )
            ktc = kt_sb[kc // 4][:, (kc % 4) * 128 : (kc % 4 + 1) * 128]
            qtc = qt_sb[qb][:, h, :]
            if td < 0:
                nc.tensor.matmul(pss, lhsT=ktc, rhs=qtc, start=True, stop=True)
                pt = ppool.tile([128, SB], BF16, name=f"pt{qb}_{h}_{kc}", tag="pt")
                nc.scalar.activation(
                    pt, pss, mybir.ActivationFunctionType.Exp, scale=SCALE
                )
            else:
                lo = 128 * td
                nc.tensor.matmul(
                    pss[:, lo:], lhsT=ktc, rhs=qtc[:, lo:], start=True, stop=True
                )
                pt = ptd[td]
                nc.scalar.activation(
                    pt[:, lo:],
                    pss[:, lo:],
                    mybir.ActivationFunctionType.Exp,
                    scale=SCALE,
                )
                # triangular strip mask (in-place on the 128-wide strip)
                nc.vector.tensor_mul(
                    pt[:, lo : lo + 128], pt[:, lo : lo + 128], maskt
                )
            pts[kc] = pt
            feed(pt)
            if kc >= 2:
                pv(kc - 2)
        pv(nkc - 2)
        pv(nkc - 1)
        # combine leftover tree levels ascending -> root
        lvls = sorted(pend)
        root = pend[lvls[0]]
        for lv in lvls[1:]:
            nt = accp.tile([128, SB], BF16, name=f"accr{qb}_{h}_{lv}", tag="accr")
            nc.vector.tensor_add(nt, pend[lv], root)
            root = nt
        # ones_mat stationary => every partition of psn gets the column-sum:
        # the softmax denominator, already broadcast.
        nc.tensor.matmul(psn, lhsT=ones_mat, rhs=root, start=True, stop=True)
        rb = spool.tile([128, SB], F32, name=f"rb{qb}_{h}", tag="rb")
        nc.vector.reciprocal_approx_fast(rb, psn)
        ao = spool.tile([128, SB], BF16, name=f"ao{qb}_{h}", tag="ao", bufs=4)
        nc.vector.tensor_mul(ao, pso, rb)
        # ao DMA on gpsimd: precedes the gather trigger on the same queue
        nc.gpsimd.dma_start(out=attn_loc[qb][h, :, :], in_=ao)
    # one AllGather per q-block covering all 4 local heads
    nc.gpsimd.collective_compute(
        "AllGather",
        mybir.AluOpType.bypass,
        replica_groups=[list(range(NCORES))],
        ins=[attn_loc[qb].opt()],
        outs=[attn_g[qb].opt()],
    )


_NC_CACHE = None


def _get_nc():
    global _NC_CACHE
    if _NC_CACHE is None:
        _NC_CACHE = _build_nc()
    return _NC_CACHE


def _prep_in_maps(x, freqs_cos, freqs_sin, wq, wk, wv, wo):
    bf = ml_dtypes.bfloat16
    x = np.asarray(x, np.float32).reshape(S, D)
    xT = np.ascontiguousarray(x.T).astype(bf)
    cos = np.asarray(freqs_cos, np.float32)  # [S, HD/2]
    sin = np.asarray(freqs_sin, np.float32)
    cos2 = np.repeat(cos.T, 2, axis=0)  # [HD, S], rows 2j,2j+1 = cos[:, j]
    sin_t = sin.T
    sin2 = np.empty((HD, S), np.float32)
    sin2[0::2] = -sin_t
    sin2[1::2] = sin_t
    p = np.arange(128)[:, None]
    c = np.arange(128)[None, :]
    maskt = (p <= c).astype(bf)  # triangular strip mask, same for every td
    ident = np.eye(128, dtype=bf)
    wq = np.asarray(wq, np.float32)
    wk = np.asarray(wk, np.float32)
    wv = np.asarray(wv, np.float32)
    wo = np.asarray(wo, np.float32)
    in_maps = []
    for i in range(NCORES):
        in_maps.append(
            {
                "xT": xT,
                "cos2": cos2.astype(bf),
                "sin2": sin2.astype(bf),
                "maskt": maskt,
                "ident": ident,
                "wq": np.ascontiguousarray(
                    wq[:, i * HQL * HD : (i + 1) * HQL * HD]
                ).astype(bf),
                "wk": np.ascontiguousarray(wk[:, i * HD : (i + 1) * HD]).astype(bf),
                "wv": np.ascontiguousarray(wv[:, i * HD : (i + 1) * HD]).astype(bf),
                "wo": np.ascontiguousarray(
                    wo[:, i * HQL * HD : (i + 1) * HQL * HD]
                ).astype(bf),
            }
        )
    return in_maps


def _install_trace_shims():
    """The container's antenv lacks axon_hooks; replicate trn_boot's ctypes
    NTFF hook so run_bass_kernel_spmd(trace=True) works. Also stub out the
    fish-bucket artifact upload (no bucket access here)."""
    import sys
    import types
    import ctypes
    import contextlib

    if "antenv.axon_hooks" not in sys.modules:
        mod = types.ModuleType("antenv.axon_hooks")
        mod._hook = None

        def set_axon_ntff_profile_hook(h):
            mod._hook = h

        def get_axon_ntff_profile_hook():
            return mod._hook

        mod.set_axon_ntff_profile_hook = set_axon_ntff_profile_hook
        mod.get_axon_ntff_profile_hook = get_axon_ntff_profile_hook
        sys.modules["antenv.axon_hooks"] = mod

        so_path = "/opt/axon/libaxon_pjrt.so"
        lib = ctypes.CDLL(so_path)
        if hasattr(lib, "axon_start_nrt_profile"):
            lib.axon_start_nrt_profile.argtypes = [
                ctypes.POINTER(ctypes.c_int64),
                ctypes.c_size_t,
            ]
            lib.axon_start_nrt_profile.restype = ctypes.c_int64
            lib.axon_stop_nrt_profile.argtypes = [ctypes.c_char_p]
            lib.axon_stop_nrt_profile.restype = ctypes.c_int64

            @contextlib.contextmanager
            def _hook(output_dir, device_ids):
                import jax

                jax.devices()
                if device_ids:
                    ids = (ctypes.c_int64 * len(device_ids))(*device_ids)
                    rc = lib.axon_start_nrt_profile(ids, len(device_ids))
                else:
                    rc = lib.axon_start_nrt_profile(None, 0)
                if rc != 0:
                    raise RuntimeError(f"axon_start_nrt_profile rc={rc}")
                try:
                    yield
                finally:
                    n = lib.axon_stop_nrt_profile(str(output_dir).encode())
                    if n <= 0:
                        print(f"WARNING: axon_stop_nrt_profile rc={n}")

            set_axon_ntff_profile_hook(_hook)

    import concourse.bass_utils as bu

    bu.upload_artifacts = lambda tmpdir: "local://" + str(tmpdir)


def run(inputs, trace=False, **kw):
    nc = _get_nc()
    if trace:
        _install_trace_shims()
    in_maps = _prep_in_maps(**inputs)
    res = run_bass_kernel_spmd(nc, in_maps, list(range(NCORES)), trace=trace, **kw)
    out = np.concatenate(
        [res.results[i]["outT"].T for i in range(NCORES)], axis=1
    )
    return out.reshape(B, S, D).astype(np.float32), res


def kernel(x, freqs_cos, freqs_sin, wq, wk, wv, wo):
    out, _ = run(
        dict(
            x=x,
            freqs_cos=freqs_cos,
            freqs_sin=freqs_sin,
            wq=wq,
            wk=wk,
            wv=wv,
            wo=wo,
        )
    )
    return out


# revision 20
# speedup vs baseline: 1.0388x; 1.0388x over previous
"""GQA attention + RoPE + causal softmax + output projection on 8 TRN2 cores.

Sharding: tensor-parallel over heads. Core i owns q-heads [4i, 4i+4) and
kv-head i (GQA group size 4 aligns exactly with HQ/8=4, HK/8=1).

Per-core pipeline (everything in transposed "feature-on-partitions" layout),
with stages A (projections+RoPE) and B (attention) interleaved per 512-wide
seq block so the per-(head,qblock) AllGathers start early and finish long
before stage D consumes them:

  for sb in 0..3:
    A(sb): Q^T/K^T/V^T projections for seq block sb (lhsT = weight chunk
      [Dc,128], rhs = x^T chunk [Dc,512] -> PSUM [feat,seq]); RoPE on Q^T/K^T
      via stream_shuffle + 2 muls + add; V^T PE-transposed to V [seq,128].
    B(qb=sb): per head, causal attention over k-chunks 0..4*(qb+1):
      scores^T [sk,128 x sq,512] = K^T-chunk (stationary) x Q^T (moving);
      p = exp(scores * 1/sqrt(hd)) on ACT. Diagonal chunks are column-trimmed
      (only sq >= 128*td is computed) into persistent zero-prefix tiles, and
      masked with one [128,128] triangular strip mask (DVE).
      Softmax denominator: exp chunks are pairwise tree-summed on DVE (bf16),
      then ONE all-ones [128,128] stationary matmul on the root broadcasts
      the column sums to every partition (vs one matmul per chunk before:
      removes 160 PE matmuls).
      out^T[128,sq] += V-chunk^T @ p (PE), 2-chunk software pipeline;
      attn^T = out^T * reciprocal_approx_fast(norm) (DVE) -> DMA (vector
      queue) -> per-(h,qb) AllGather [128,512]->[1024,512] on gpsimd.
  D: out^T column shard: lhsT = wo chunk, rhs = gathered attn^T chunk for
     seq block g (depends only on gathers (h, qb=g) - all long done),
     accumulated over all 4096 contraction rows. Host transposes + concats.

PSUM tags are shared across stages (8 banks total): A accumulators
psq0-3/psk/psv, B score tiles rotate over psq0-3, B out/norm use psk/psv and
extra0, V-transpose uses its own bank, D accumulators rotate over psq0-3.

Matmul operands are bf16 (1 cycle/row on PE); accumulation is fp32 in PSUM;
softmax denominator and normalization stay fp32 after the bf16 chunk tree.
"""

import numpy as np
import ml_dtypes

import concourse.bass as bass
import concourse.mybir as mybir
import concourse.tile as tile
from concourse import bacc
from concourse.bass_utils import run_bass_kernel_spmd

# Problem dims (hardcoded per contract)
B, S, D = 1, 2048, 4096
HQ, HK, HD = 32, 8, 128
NCORES = 8
HQL = HQ // NCORES          # 4 local q heads
SB = 512                    # seq block (matmul moving free dim)
NB = S // SB                # 4 seq blocks
NC_ = D // 128              # 32 contraction chunks for D
SCALE = 1.0 / float(np.sqrt(HD))

F32 = mybir.dt.float32
BF16 = mybir.dt.bfloat16

# stream_shuffle mask: swap adjacent pairs within each 32-partition quadrant
SWAP_MASK = [(i ^ 1) for i in range(32)]


def _build_nc():
    nc = bacc.Bacc(
        "TRN2", target_bir_lowering=False, debug=False, num_devices=NCORES
    )

    io = {}
    io["xT"] = nc.dram_tensor("xT", [D, S], BF16, kind="ExternalInput")
    io["wq"] = nc.dram_tensor("wq", [D, HQL * HD], BF16, kind="ExternalInput")
    io["wk"] = nc.dram_tensor("wk", [D, HD], BF16, kind="ExternalInput")
    io["wv"] = nc.dram_tensor("wv", [D, HD], BF16, kind="ExternalInput")
    io["wo"] = nc.dram_tensor("wo", [D, HQL * HD], BF16, kind="ExternalInput")
    io["cos2"] = nc.dram_tensor("cos2", [HD, S], BF16, kind="ExternalInput")
    io["sin2"] = nc.dram_tensor("sin2", [HD, S], BF16, kind="ExternalInput")
    io["maskt"] = nc.dram_tensor("maskt", [128, 128], BF16, kind="ExternalInput")
    io["ident"] = nc.dram_tensor("ident", [128, 128], BF16, kind="ExternalInput")
    io["outT"] = nc.dram_tensor("outT", [HQL * HD, S], F32, kind="ExternalOutput")

    with tile.TileContext(nc) as tc:
        _body(tc, io)
    nc.compile()
    return nc


def _body(tc, io):
    nc = tc.nc
    from contextlib import ExitStack

    ctx = ExitStack()
    with ctx:
        consts = ctx.enter_context(tc.tile_pool(name="consts", bufs=1))
        qkv = ctx.enter_context(tc.tile_pool(name="qkv", bufs=1))
        dram = ctx.enter_context(tc.tile_pool(name="dram", bufs=1, space="DRAM"))
        wpool = ctx.enter_context(tc.tile_pool(name="wpool", bufs=1))
        xpool = ctx.enter_context(tc.tile_pool(name="xpool", bufs=12))
        rpool = ctx.enter_context(tc.tile_pool(name="rpool", bufs=3))
        ppool = ctx.enter_context(tc.tile_pool(name="ppool", bufs=6))
        accp = ctx.enter_context(tc.tile_pool(name="accp", bufs=2))
        spool = ctx.enter_context(tc.tile_pool(name="spool", bufs=2))
        apool = ctx.enter_context(tc.tile_pool(name="apool", bufs=8))
        opool = ctx.enter_context(tc.tile_pool(name="opool", bufs=4))
        ps = ctx.enter_context(tc.tile_pool(name="ps", bufs=1, space="PSUM"))

        # ---- constants ----
        cos2 = consts.tile([HD, S], BF16)
        sin2 = consts.tile([HD, S], BF16)
        ident = consts.tile([128, 128], BF16)
        maskt = consts.tile([128, 128], BF16)
        ones_mat = consts.tile([128, 128], BF16)
        nc.vector.memset(ones_mat, 1.0)

        # ---- persistent per-core tensors ----
        qt_sb = [
            qkv.tile([HD, HQL, SB], BF16, name=f"qt{sb}") for sb in range(NB)
        ]
        kt_sb = [qkv.tile([HD, SB], BF16, name=f"kt{sb}") for sb in range(NB)]
        vs_sb = [
            qkv.tile([128, SB // 128, HD], BF16, name=f"vs{sb}") for sb in range(NB)
        ]
        # persistent zero-prefix exp tiles for the 4 diagonal chunk offsets;
        # prefix [0, 128*td) is zeroed once and never written again, so the
        # trimmed exp + full-width PV/denominator accumulation stay correct.
        ptd = [qkv.tile([128, SB], BF16, name=f"ptd{td}") for td in range(4)]
        for td in range(1, 4):
            nc.vector.memset(ptd[td][:, : 128 * td], 0.0)

        # per-qblock bounce + gather buffers: one AllGather per q-block
        # covering all 4 local heads (h-major rows), issued at each B(qb)
        # end -> overlaps the remaining A/B blocks and stage D. Gathered
        # rows are core-major then head: row (i*HQL + j)*128 + p.
        attn_loc = [
            dram.tile([HQL, HD, SB], BF16, name=f"aloc{qb}") for qb in range(NB)
        ]
        attn_g = [
            dram.tile(
                [NCORES * HQL * HD, SB], BF16, name=f"ag{qb}", addr_space="Shared"
            )
            for qb in range(NB)
        ]

        # ---- weight loads: first chunks spread across queues so the PE
        # starts ASAP; bulk on gpsimd (done during A(0), before collectives)
        wq_sb = wpool.tile([128, NC_, HQL * HD], BF16)
        wk_sb = wpool.tile([128, NC_, HD], BF16)
        wv_sb = wpool.tile([128, NC_, HD], BF16)
        for t in range(HQL):
            eng = [nc.scalar, nc.scalar, nc.sync, nc.sync][t]
            eng.dma_start(
                out=wq_sb[:, 0, t * 128 : (t + 1) * 128],
                in_=io["wq"][0:128, t * 128 : (t + 1) * 128],
            )
        nc.gpsimd.dma_start(out=wk_sb[:, 0, :], in_=io["wk"][0:128, :])
        nc.gpsimd.dma_start(out=wv_sb[:, 0, :], in_=io["wv"][0:128, :])
        nc.gpsimd.dma_start(out=maskt, in_=io["maskt"][:, :])
        nc.gpsimd.dma_start(out=ident, in_=io["ident"][:, :])
        for c in range(1, 4):
            sl = slice(c * 128, (c + 1) * 128)
            nc.gpsimd.dma_start(out=wq_sb[:, c, :], in_=io["wq"][sl, :])
            nc.gpsimd.dma_start(out=wk_sb[:, c, :], in_=io["wk"][sl, :])
            nc.gpsimd.dma_start(out=wv_sb[:, c, :], in_=io["wv"][sl, :])
        for c4 in range(1, NC_ // 4):
            sl = slice(c4 * 4 * 128, (c4 * 4 + 4) * 128)
            nc.gpsimd.dma_start(
                out=wq_sb[:, c4 * 4 : c4 * 4 + 4, :],
                in_=io["wq"][sl, :].rearrange("(c p) n -> p c n", p=128),
            )
            nc.gpsimd.dma_start(
                out=wk_sb[:, c4 * 4 : c4 * 4 + 4, :],
                in_=io["wk"][sl, :].rearrange("(c p) n -> p c n", p=128),
            )
            nc.gpsimd.dma_start(
                out=wv_sb[:, c4 * 4 : c4 * 4 + 4, :],
                in_=io["wv"][sl, :].rearrange("(c p) n -> p c n", p=128),
            )
        # bulk constants after the weights on gpsimd: needed first at A(0)'s
        # end (rope) - keeps the scalar queue free for xt/evictions
        nc.gpsimd.dma_start(out=cos2, in_=io["cos2"][:, :])
        nc.gpsimd.dma_start(out=sin2, in_=io["sin2"][:, :])

        wo_sb = wpool.tile([128, NC_, HQL * HD], BF16)

        # xt chunk-pair loader: [128, 2, SB] tiles (chunks 2cp, 2cp+1) on the
        # scalar queue (the sync queue is kept clear: the collectives' D2D
        # transfers run there and would stall compute-critical loads).
        xts = {}

        def emit_xt(sb, cp, eng):
            t = xpool.tile([128, 2, SB], BF16, tag="xt")
            ssl = slice(sb * SB, (sb + 1) * SB)
            sl = slice(cp * 256, (cp + 1) * 256)
            eng.dma_start(
                out=t, in_=io["xT"][sl, ssl].rearrange("(c p) n -> p c n", p=128)
            )
            xts[(sb, cp)] = t

        emit_xt(0, 0, nc.sync)  # very first pair in parallel with wq chunks

        # =============== interleaved stage A(sb) + stage B(qb=sb) ===========
        for sb in range(NB):
            _stage_a_block(nc, tc, io, sb, ps, xpool, rpool, xts, emit_xt,
                           (wq_sb, wk_sb, wv_sb),
                           cos2, sin2, ident, qt_sb, kt_sb, vs_sb)
            if sb == 0:
                # wo loads fill DMA idle time from here on (gpsimd queue,
                # after the qkv weights and before most collectives)
                for c4 in range(NC_ // 4):
                    sl = slice(c4 * 4 * 128, (c4 * 4 + 4) * 128)
                    nc.gpsimd.dma_start(
                        out=wo_sb[:, c4 * 4 : c4 * 4 + 4, :],
                        in_=io["wo"][sl, :].rearrange("(c p) n -> p c n", p=128),
                    )
            if sb + 1 < NB:
                # prefetch the first 12 chunks of the next block ahead of
                # B(qb)'s exp work in the scalar queue
                for cp in range(6):
                    emit_xt(sb + 1, cp, nc.scalar)
            _stage_b_block(nc, tc, sb, ps, ppool, accp, spool, maskt, ones_mat,
                           qt_sb, kt_sb, vs_sb, ptd, attn_loc, attn_g)

        # ================= Stage D: out = attn @ wo (column shard) =========
        for g in range(NB):
            gsl = slice(g * SB, (g + 1) * SB)
            ps_d = [
                ps.tile([128, SB], F32, name=f"psd{g}_{n}", tag=f"psq{n}")
                for n in range(HQL)
            ]
            for j in range(HQL):
                for i in range(NCORES):
                    c = i * HQL + j
                    # at loads on sync: the scheduler hoists them as far as
                    # the apool ring allows and they head-block their queue
                    # waiting on the gather semaphore - sync has nothing
                    # else compute-critical, so that head-block is free
                    at = apool.tile([128, SB], BF16, tag="at")
                    nc.sync.dma_start(
                        out=at, in_=attn_g[g][c * 128 : (c + 1) * 128, :]
                    )
                    first = j == 0 and i == 0
                    last = j == HQL - 1 and i == NCORES - 1
                    for n in range(HQL):
                        nc.tensor.matmul(
                            ps_d[n],
                            lhsT=wo_sb[:, c, n * 128 : (n + 1) * 128],
                            rhs=at,
                            start=first,
                            stop=last,
                        )
            for n in range(HQL):
                # alternate ACT/DVE for the PSUM evictions: halves the
                # serialized copy tail after each g-group's last matmul
                ot = opool.tile([128, SB], F32, name=f"ot{g}_{n}", tag="ot")
                if n % 2 == 0:
                    nc.scalar.copy(ot, ps_d[n])
                else:
                    nc.vector.tensor_copy(ot, ps_d[n])
                nc.gpsimd.dma_start(
                    out=io["outT"][n * 128 : (n + 1) * 128, gsl], in_=ot
                )


def _stage_a_block(nc, tc, io, sb, ps, xpool, rpool, xts, emit_xt, weights,
                   cos2, sin2, ident, qt_sb, kt_sb, vs_sb):
    """Projections + RoPE for seq block sb."""
    ssl = slice(sb * SB, (sb + 1) * SB)
    ps_q = [
        ps.tile([128, SB], F32, name=f"psq{t}_{sb}", tag=f"psq{t}")
        for t in range(HQL)
    ]
    ps_k = ps.tile([128, SB], F32, name=f"psk_{sb}", tag="psk")
    ps_v = ps.tile([128, SB], F32, name=f"psv_{sb}", tag="psv")
    wq_sb, wk_sb, wv_sb = weights
    for c in range(NC_):
        if c % 2 == 0:
            if (sb, c // 2) not in xts:
                emit_xt(sb, c // 2, nc.scalar)
            xt2 = xts.pop((sb, c // 2))
        xt = xt2[:, c % 2, :]
        first, last = c == 0, c == NC_ - 1
        for t in range(HQL):
            nc.tensor.matmul(
                ps_q[t],
                lhsT=wq_sb[:, c, t * 128 : (t + 1) * 128],
                rhs=xt,
                start=first,
                stop=last,
            )
        nc.tensor.matmul(
            ps_k, lhsT=wk_sb[:, c, :], rhs=xt, start=first, stop=last
        )
        nc.tensor.matmul(
            ps_v, lhsT=wv_sb[:, c, :], rhs=xt, start=first, stop=last
        )

    # PSUM evictions (ACT copies, bf16 cast). k first for sb==0 so B(0)'s
    # first scores aren't gated on the whole rope chain; q0 first otherwise.
    qc_k = rpool.tile([128, SB], BF16, name=f"qck{sb}", tag="qck")
    qc_q = [
        rpool.tile([128, SB], BF16, name=f"qc{sb}_{t}", tag=f"qc{t}")
        for t in range(HQL)
    ]

    def rope_dve(qc, dst, idx):
        sw = rpool.tile([128, SB], BF16, name=f"sw{idx}", tag="sw")
        nc.vector.stream_shuffle(sw, qc, SWAP_MASK)
        t1 = rpool.tile([128, SB], BF16, name=f"t1{idx}", tag="t1")
        nc.vector.tensor_mul(t1, qc, cos2[:, ssl])
        t2 = rpool.tile([128, SB], BF16, name=f"t2{idx}", tag="t2")
        nc.vector.tensor_mul(t2, sw, sin2[:, ssl])
        nc.vector.tensor_add(dst, t1, t2)

    if sb == 0:
        nc.scalar.copy(qc_k, ps_k)
        rope_dve(qc_k, kt_sb[sb], f"k{sb}")
        nc.scalar.copy(qc_q[0], ps_q[0])
        rope_dve(qc_q[0], qt_sb[sb][:, 0, :], f"q{sb}_0")
    else:
        nc.scalar.copy(qc_q[0], ps_q[0])
        rope_dve(qc_q[0], qt_sb[sb][:, 0, :], f"q{sb}_0")
        nc.scalar.copy(qc_k, ps_k)
        rope_dve(qc_k, kt_sb[sb], f"k{sb}")
    for t in range(1, HQL):
        nc.scalar.copy(qc_q[t], ps_q[t])
        rope_dve(qc_q[t], qt_sb[sb][:, t, :], f"q{sb}_{t}")

    # V^T -> V via XBAR DMA transpose per 128-col chunk (SBUF->SBUF, frees
    # the PE and a PSUM bank vs the PE-transpose path). The triggers follow
    # the vts ACT copy on the same (scalar) queue.
    vts = rpool.tile([128, SB], BF16, name=f"vts{sb}", tag="vts")
    nc.scalar.copy(vts, ps_v)
    for u in range(SB // 128):
        nc.scalar.dma_start(
            out=vs_sb[sb][:, u, :],
            in_=vts[:, u * 128 : (u + 1) * 128],
            transpose=True,
        )


def _stage_b_block(nc, tc, qb, ps, ppool, accp, spool, maskt, ones_mat,
                   qt_sb, kt_sb, vs_sb, ptd, attn_loc, attn_g):
    """Causal attention for q-block qb over k-chunks 0..4*(qb+1)."""
    nkc = 4 * (qb + 1)
    for h in range(HQL):
        pso = ps.tile(
            [128, SB], F32, name=f"pso{qb}_{h}", tag=("psk" if h % 2 == 0 else "psv")
        )
        psn = ps.tile(
            [128, SB], F32, name=f"psn{qb}_{h}",
            tag=("extra0" if h % 2 == 0 else "extra1"),
        )
        pts = {}
        pend = {}  # binary-counter pairwise tree accumulation on DVE

        def feed(t, _s=[0]):
            lvl = 0
            while lvl in pend:
                prev = pend.pop(lvl)
                _s[0] += 1
                nt = accp.tile(
                    [128, SB], BF16, name=f"acc{qb}_{h}_{_s[0]}", tag=f"acc{lvl}"
                )
                nc.vector.tensor_add(nt, prev, t)
                t = nt
                lvl += 1
            pend[lvl] = t

        def pv(kc):
            nc.tensor.matmul(
                pso,
                lhsT=vs_sb[kc // 4][:, kc % 4, :],
                rhs=pts.pop(kc),
                start=kc == 0,
                stop=kc == nkc - 1,
            )

        for kc in range(nkc):
            td = kc - 4 * qb
            pss = ps.tile(
                [128, SB], F32, name=f"pss{qb}_{h}_{kc}", tag=f"psq{kc % 4}"
            )
            ktc = kt_sb[kc // 4][:, (kc % 4) * 128 : (kc % 4 + 1) * 128]
            qtc = qt_sb[qb][:, h, :]
            if td < 0:
                nc.tensor.matmul(pss, lhsT=ktc, rhs=qtc, start=True, stop=True)
                pt = ppool.tile([128, SB], BF16, name=f"pt{qb}_{h}_{kc}", tag="pt")
                nc.scalar.activation(
                    pt, pss, mybir.ActivationFunctionType.Exp, scale=SCALE
                )
            else:
                lo = 128 * td
                nc.tensor.matmul(
                    pss[:, lo:], lhsT=ktc, rhs=qtc[:, lo:], start=True, stop=True
                )
                pt = ptd[td]
                nc.scalar.activation(
                    pt[:, lo:],
                    pss[:, lo:],
                    mybir.ActivationFunctionType.Exp,
                    scale=SCALE,
                )
                # triangular strip mask (in-place on the 128-wide strip)
                nc.vector.tensor_mul(
                    pt[:, lo : lo + 128], pt[:, lo : lo + 128], maskt
                )
            pts[kc] = pt
            feed(pt)
            if kc >= 2:
                pv(kc - 2)
        pv(nkc - 2)
        pv(nkc - 1)
        # combine leftover tree levels ascending -> root
        lvls = sorted(pend)
        root = pend[lvls[0]]
        for lv in lvls[1:]:
            nt = accp.tile([128, SB], BF16, name=f"accr{qb}_{h}_{lv}", tag="accr")
            nc.vector.tensor_add(nt, pend[lv], root)
            root = nt
        # ones_mat stationary => every partition of psn gets the column-sum:
        # the softmax denominator, already broadcast.
        nc.tensor.matmul(psn, lhsT=ones_mat, rhs=root, start=True, stop=True)
        rb = spool.tile([128, SB], F32, name=f"rb{qb}_{h}", tag="rb")
        nc.vector.reciprocal_approx_fast(rb, psn)
        ao = spool.tile([128, SB], BF16, name=f"ao{qb}_{h}", tag="ao", bufs=4)
        nc.vector.tensor_mul(ao, pso, rb)
        # ao DMA on gpsimd: precedes the gather trigger on the same queue
        nc.gpsimd.dma_start(out=attn_loc[qb][h, :, :], in_=ao)
    # one AllGather per q-block covering all 4 local heads
    nc.gpsimd.collective_compute(
        "AllGather",
        mybir.AluOpType.bypass,
        replica_groups=[list(range(NCORES))],
        ins=[attn_loc[qb].opt()],
        outs=[attn_g[qb].opt()],
    )


_NC_CACHE = None


def _get_nc():
    global _NC_CACHE
    if _NC_CACHE is None:
        _NC_CACHE = _build_nc()
    return _NC_CACHE


def _prep_in_maps(x, freqs_cos, freqs_sin, wq, wk, wv, wo):
    bf = ml_dtypes.bfloat16
    x = np.asarray(x, np.float32).reshape(S, D)
    xT = np.ascontiguousarray(x.T).astype(bf)
    cos = np.asarray(freqs_cos, np.float32)  # [S, HD/2]
    sin = np.asarray(freqs_sin, np.float32)
    cos2 = np.repeat(cos.T, 2, axis=0)  # [HD, S], rows 2j,2j+1 = cos[:, j]
    sin_t = sin.T
    sin2 = np.empty((HD, S), np.float32)
    sin2[0::2] = -sin_t
    sin2[1::2] = sin_t
    p = np.arange(128)[:, None]
    c = np.arange(128)[None, :]
    maskt = (p <= c).astype(bf)  # triangular strip mask, same for every td
    ident = np.eye(128, dtype=bf)
    wq = np.asarray(wq, np.float32)
    wk = np.asarray(wk, np.float32)
    wv = np.asarray(wv, np.float32)
    wo = np.asarray(wo, np.float32)
    in_maps = []
    for i in range(NCORES):
        in_maps.append(
            {
                "xT": xT,
                "cos2": cos2.astype(bf),
                "sin2": sin2.astype(bf),
                "maskt": maskt,
                "ident": ident,
                "wq": np.ascontiguousarray(
                    wq[:, i * HQL * HD : (i + 1) * HQL * HD]
                ).astype(bf),
                "wk": np.ascontiguousarray(wk[:, i * HD : (i + 1) * HD]).astype(bf),
                "wv": np.ascontiguousarray(wv[:, i * HD : (i + 1) * HD]).astype(bf),
                "wo": np.ascontiguousarray(
                    wo[:, i * HQL * HD : (i + 1) * HQL * HD]
                ).astype(bf),
            }
        )
    return in_maps


def _install_trace_shims():
    """The container's antenv lacks axon_hooks; replicate trn_boot's ctypes
    NTFF hook so run_bass_kernel_spmd(trace=True) works. Also stub out the
    fish-bucket artifact upload (no bucket access here)."""
    import sys
    import types
    import ctypes
    import contextlib

    if "antenv.axon_hooks" not in sys.modules:
        mod = types.ModuleType("antenv.axon_hooks")
        mod._hook = None

        def set_axon_ntff_profile_hook(h):
            mod._hook = h

        def get_axon_ntff_profile_hook():
            return mod._hook

        mod.set_axon_ntff_profile_hook = set_axon_ntff_profile_hook
        mod.get_axon_ntff_profile_hook = get_axon_ntff_profile_hook
        sys.modules["antenv.axon_hooks"] = mod

        so_path = "/opt/axon/libaxon_pjrt.so"
        lib = ctypes.CDLL(so_path)
        if hasattr(lib, "axon_start_nrt_profile"):
            lib.axon_start_nrt_profile.argtypes = [
                ctypes.POINTER(ctypes.c_int64),
                ctypes.c_size_t,
            ]
            lib.axon_start_nrt_profile.restype = ctypes.c_int64
            lib.axon_stop_nrt_profile.argtypes = [ctypes.c_char_p]
            lib.axon_stop_nrt_profile.restype = ctypes.c_int64

            @contextlib.contextmanager
            def _hook(output_dir, device_ids):
                import jax

                jax.devices()
                if device_ids:
                    ids = (ctypes.c_int64 * len(device_ids))(*device_ids)
                    rc = lib.axon_start_nrt_profile(ids, len(device_ids))
                else:
                    rc = lib.axon_start_nrt_profile(None, 0)
                if rc != 0:
                    raise RuntimeError(f"axon_start_nrt_profile rc={rc}")
                try:
                    yield
                finally:
                    n = lib.axon_stop_nrt_profile(str(output_dir).encode())
                    if n <= 0:
                        print(f"WARNING: axon_stop_nrt_profile rc={n}")

            set_axon_ntff_profile_hook(_hook)

    import concourse.bass_utils as bu

    bu.upload_artifacts = lambda tmpdir: "local://" + str(tmpdir)


def run(inputs, trace=False, **kw):
    nc = _get_nc()
    if trace:
        _install_trace_shims()
    in_maps = _prep_in_maps(**inputs)
    res = run_bass_kernel_spmd(nc, in_maps, list(range(NCORES)), trace=trace, **kw)
    out = np.concatenate(
        [res.results[i]["outT"].T for i in range(NCORES)], axis=1
    )
    return out.reshape(B, S, D).astype(np.float32), res


def kernel(x, freqs_cos, freqs_sin, wq, wk, wv, wo):
    out, _ = run(
        dict(
            x=x,
            freqs_cos=freqs_cos,
            freqs_sin=freqs_sin,
            wq=wq,
            wk=wk,
            wv=wv,
            wo=wo,
        )
    )
    return out


# revision 27
# speedup vs baseline: 1.0899x; 1.0492x over previous
"""GQA attention + RoPE + causal softmax + output projection on 8 TRN2 cores.

Sharding: tensor-parallel over heads. Core i owns q-heads [4i, 4i+4) and
kv-head i (GQA group size 4 aligns exactly with HQ/8=4, HK/8=1).

Per-core pipeline (everything in transposed "feature-on-partitions" layout),
with stages A (projections+RoPE) and B (attention) interleaved per 512-wide
seq block so the per-(head,qblock) AllGathers start early and finish long
before stage D consumes them:

  for sb in 0..3:
    A(sb): Q^T/K^T/V^T projections for seq block sb (lhsT = weight chunk
      [Dc,128], rhs = x^T chunk [Dc,512] -> PSUM [feat,seq]); RoPE on Q^T/K^T
      via stream_shuffle + 2 muls + add; V^T PE-transposed to V [seq,128].
    B(qb=sb): per head, causal attention over k-chunks 0..4*(qb+1):
      scores^T [sk,128 x sq,512] = K^T-chunk (stationary) x Q^T (moving);
      p = exp(scores * 1/sqrt(hd)) on ACT. Diagonal chunks are column-trimmed
      (only sq >= 128*td is computed) into persistent zero-prefix tiles, and
      masked with one [128,128] triangular strip mask (DVE).
      Softmax denominator: exp chunks are pairwise tree-summed on DVE (bf16),
      then ONE all-ones [128,128] stationary matmul on the root broadcasts
      the column sums to every partition (vs one matmul per chunk before:
      removes 160 PE matmuls).
      out^T[128,sq] += V-chunk^T @ p (PE), 2-chunk software pipeline;
      attn^T = out^T * reciprocal_approx_fast(norm) (DVE) -> DMA (vector
      queue) -> per-(h,qb) AllGather [128,512]->[1024,512] on gpsimd.
  D: out^T column shard: lhsT = wo chunk, rhs = gathered attn^T chunk for
     seq block g (depends only on gathers (h, qb=g) - all long done),
     accumulated over all 4096 contraction rows. Host transposes + concats.

PSUM tags are shared across stages (8 banks total): A accumulators
psq0-3/psk/psv, B score tiles rotate over psq0-3, B out/norm use psk/psv and
extra0, V-transpose uses its own bank, D accumulators rotate over psq0-3.

Matmul operands are bf16 (1 cycle/row on PE); accumulation is fp32 in PSUM;
softmax denominator and normalization stay fp32 after the bf16 chunk tree.
"""

import numpy as np
import ml_dtypes

import concourse.bass as bass
import concourse.mybir as mybir
import concourse.tile as tile
from concourse import bacc
from concourse.bass_utils import run_bass_kernel_spmd

# Problem dims (hardcoded per contract)
B, S, D = 1, 2048, 4096
HQ, HK, HD = 32, 8, 128
NCORES = 8
HQL = HQ // NCORES          # 4 local q heads
SB = 512                    # seq block (matmul moving free dim)
NB = S // SB                # 4 seq blocks
NC_ = D // 128              # 32 contraction chunks for D
SCALE = 1.0 / float(np.sqrt(HD))

F32 = mybir.dt.float32
BF16 = mybir.dt.bfloat16

# stream_shuffle mask: swap adjacent pairs within each 32-partition quadrant
SWAP_MASK = [(i ^ 1) for i in range(32)]


def _build_nc():
    nc = bacc.Bacc(
        "TRN2", target_bir_lowering=False, debug=False, num_devices=NCORES
    )

    io = {}
    io["xT"] = nc.dram_tensor("xT", [D, S], BF16, kind="ExternalInput")
    io["wq"] = nc.dram_tensor("wq", [D, HQL * HD], BF16, kind="ExternalInput")
    io["wk"] = nc.dram_tensor("wk", [D, HD], BF16, kind="ExternalInput")
    io["wv"] = nc.dram_tensor("wv", [D, HD], BF16, kind="ExternalInput")
    io["wo"] = nc.dram_tensor("wo", [D, HQL * HD], BF16, kind="ExternalInput")
    io["cos2"] = nc.dram_tensor("cos2", [HD, S], BF16, kind="ExternalInput")
    io["sin2"] = nc.dram_tensor("sin2", [HD, S], BF16, kind="ExternalInput")
    io["maskt"] = nc.dram_tensor("maskt", [128, 128], BF16, kind="ExternalInput")
    io["ident"] = nc.dram_tensor("ident", [128, 128], BF16, kind="ExternalInput")
    io["outT"] = nc.dram_tensor("outT", [HQL * HD, S], F32, kind="ExternalOutput")

    with tile.TileContext(nc) as tc:
        _body(tc, io)
    nc.compile()
    return nc


def _body(tc, io):
    nc = tc.nc
    from contextlib import ExitStack

    ctx = ExitStack()
    with ctx:
        consts = ctx.enter_context(tc.tile_pool(name="consts", bufs=1))
        qkv = ctx.enter_context(tc.tile_pool(name="qkv", bufs=1))
        dram = ctx.enter_context(tc.tile_pool(name="dram", bufs=1, space="DRAM"))
        wpool = ctx.enter_context(tc.tile_pool(name="wpool", bufs=1))
        xpool = ctx.enter_context(tc.tile_pool(name="xpool", bufs=12))
        rpool = ctx.enter_context(tc.tile_pool(name="rpool", bufs=3))
        ppool = ctx.enter_context(tc.tile_pool(name="ppool", bufs=6))
        accp = ctx.enter_context(tc.tile_pool(name="accp", bufs=2))
        spool = ctx.enter_context(tc.tile_pool(name="spool", bufs=2))
        apool = ctx.enter_context(tc.tile_pool(name="apool", bufs=8))
        opool = ctx.enter_context(tc.tile_pool(name="opool", bufs=4))
        ps = ctx.enter_context(tc.tile_pool(name="ps", bufs=1, space="PSUM"))

        # ---- constants ----
        cos2 = consts.tile([HD, S], BF16)
        sin2 = consts.tile([HD, S], BF16)
        ident = consts.tile([128, 128], BF16)
        maskt = consts.tile([128, 128], BF16)
        ones_mat = consts.tile([128, 128], BF16)
        nc.vector.memset(ones_mat, 1.0)

        # PE warm-up: ~48 tiny matmuls with no data dependencies fill the
        # initial weight-DMA wait and release the HAM clock gate (4096-cycle
        # activity window) before the first real matmul arrives.
        warm = ps.tile([128, SB], F32, name="warm", tag="extra0")
        for _ in range(48):
            nc.tensor.matmul(
                warm[0:64, 0:64],
                lhsT=ones_mat[:, 0:64],
                rhs=ones_mat[:, 0:64],
                start=True,
                stop=True,
            )

        # ---- persistent per-core tensors ----
        qt_sb = [
            qkv.tile([HD, HQL, SB], BF16, name=f"qt{sb}") for sb in range(NB)
        ]
        kt_sb = [qkv.tile([HD, SB], BF16, name=f"kt{sb}") for sb in range(NB)]
        vs_sb = [
            qkv.tile([128, SB // 128, HD], BF16, name=f"vs{sb}") for sb in range(NB)
        ]
        # persistent zero-prefix exp tiles for the 4 diagonal chunk offsets;
        # prefix [0, 128*td) is zeroed once and never written again, so the
        # trimmed exp + full-width PV/denominator accumulation stay correct.
        ptd = [qkv.tile([128, SB], BF16, name=f"ptd{td}") for td in range(4)]
        for td in range(1, 4):
            nc.vector.memset(ptd[td][:, : 128 * td], 0.0)

        # per-qblock bounce + gather buffers: one AllGather per q-block
        # covering all 4 local heads (h-major rows), issued at each B(qb)
        # end -> overlaps the remaining A/B blocks and stage D. Gathered
        # rows are core-major then head: row (i*HQL + j)*128 + p.
        attn_loc = [
            dram.tile([HQL, HD, SB], BF16, name=f"aloc{qb}") for qb in range(NB)
        ]
        attn_g = [
            dram.tile(
                [NCORES * HQL * HD, SB], BF16, name=f"ag{qb}", addr_space="Shared"
            )
            for qb in range(NB)
        ]

        # ---- weight loads: first chunks spread across queues so the PE
        # starts ASAP; bulk on gpsimd (done during A(0), before collectives)
        wq_sb = wpool.tile([128, NC_, HQL * HD], BF16)
        wk_sb = wpool.tile([128, NC_, HD], BF16)
        wv_sb = wpool.tile([128, NC_, HD], BF16)
        for t in range(HQL):
            eng = [nc.scalar, nc.scalar, nc.sync, nc.sync][t]
            eng.dma_start(
                out=wq_sb[:, 0, t * 128 : (t + 1) * 128],
                in_=io["wq"][0:128, t * 128 : (t + 1) * 128],
            )
        nc.gpsimd.dma_start(out=wk_sb[:, 0, :], in_=io["wk"][0:128, :])
        nc.gpsimd.dma_start(out=wv_sb[:, 0, :], in_=io["wv"][0:128, :])
        nc.gpsimd.dma_start(out=maskt, in_=io["maskt"][:, :])
        nc.gpsimd.dma_start(out=ident, in_=io["ident"][:, :])
        for c in range(1, 4):
            sl = slice(c * 128, (c + 1) * 128)
            nc.gpsimd.dma_start(out=wq_sb[:, c, :], in_=io["wq"][sl, :])
            nc.gpsimd.dma_start(out=wk_sb[:, c, :], in_=io["wk"][sl, :])
            nc.gpsimd.dma_start(out=wv_sb[:, c, :], in_=io["wv"][sl, :])
        for c4 in range(1, NC_ // 4):
            sl = slice(c4 * 4 * 128, (c4 * 4 + 4) * 128)
            nc.gpsimd.dma_start(
                out=wq_sb[:, c4 * 4 : c4 * 4 + 4, :],
                in_=io["wq"][sl, :].rearrange("(c p) n -> p c n", p=128),
            )
            nc.gpsimd.dma_start(
                out=wk_sb[:, c4 * 4 : c4 * 4 + 4, :],
                in_=io["wk"][sl, :].rearrange("(c p) n -> p c n", p=128),
            )
            nc.gpsimd.dma_start(
                out=wv_sb[:, c4 * 4 : c4 * 4 + 4, :],
                in_=io["wv"][sl, :].rearrange("(c p) n -> p c n", p=128),
            )
        # bulk constants after the weights on gpsimd: needed first at A(0)'s
        # end (rope) - keeps the scalar queue free for xt/evictions
        nc.gpsimd.dma_start(out=cos2, in_=io["cos2"][:, :])
        nc.gpsimd.dma_start(out=sin2, in_=io["sin2"][:, :])

        wo_sb = wpool.tile([128, NC_, HQL * HD], BF16)

        # xt chunk-pair loader: [128, 2, SB] tiles (chunks 2cp, 2cp+1) on the
        # scalar queue (the sync queue is kept clear: the collectives' D2D
        # transfers run there and would stall compute-critical loads).
        xts = {}

        def emit_xt(sb, cp, eng):
            t = xpool.tile([128, 2, SB], BF16, tag="xt")
            ssl = slice(sb * SB, (sb + 1) * SB)
            sl = slice(cp * 256, (cp + 1) * 256)
            eng.dma_start(
                out=t, in_=io["xT"][sl, ssl].rearrange("(c p) n -> p c n", p=128)
            )
            xts[(sb, cp)] = t

        emit_xt(0, 0, nc.sync)  # very first pair in parallel with wq chunks

        # =============== interleaved stage A(sb) + stage B(qb=sb) ===========
        for sb in range(NB):
            deferred, transp = _stage_a_block(
                nc, tc, io, sb, ps, xpool, rpool, xts, emit_xt,
                (wq_sb, wk_sb, wv_sb),
                cos2, sin2, ident, qt_sb, kt_sb, vs_sb)
            if sb == 0:
                # wo loads fill DMA idle time from here on (gpsimd queue,
                # after the qkv weights and before most collectives)
                for c4 in range(NC_ // 4):
                    sl = slice(c4 * 4 * 128, (c4 * 4 + 4) * 128)
                    nc.gpsimd.dma_start(
                        out=wo_sb[:, c4 * 4 : c4 * 4 + 4, :],
                        in_=io["wo"][sl, :].rearrange("(c p) n -> p c n", p=128),
                    )
            if sb + 1 < NB:
                # prefetch the first 12 chunks of the next block ahead of
                # B(qb)'s exp work in the scalar queue
                for cp in range(6):
                    emit_xt(sb + 1, cp, nc.scalar)
            _stage_b_block(nc, tc, sb, ps, ppool, accp, spool, maskt, ones_mat,
                           qt_sb, kt_sb, vs_sb, ptd, attn_loc, attn_g,
                           deferred, transp)

        # ================= Stage D: out = attn @ wo (column shard) =========
        for g in range(NB):
            gsl = slice(g * SB, (g + 1) * SB)
            ps_d = [
                ps.tile([128, SB], F32, name=f"psd{g}_{n}", tag=f"psq{n}")
                for n in range(HQL)
            ]
            for j in range(HQL):
                for i in range(NCORES):
                    c = i * HQL + j
                    # at loads on sync: the scheduler hoists them as far as
                    # the apool ring allows and they head-block their queue
                    # waiting on the gather semaphore - sync has nothing
                    # else compute-critical, so that head-block is free
                    at = apool.tile([128, SB], BF16, tag="at")
                    nc.sync.dma_start(
                        out=at, in_=attn_g[g][c * 128 : (c + 1) * 128, :]
                    )
                    first = j == 0 and i == 0
                    last = j == HQL - 1 and i == NCORES - 1
                    for n in range(HQL):
                        nc.tensor.matmul(
                            ps_d[n],
                            lhsT=wo_sb[:, c, n * 128 : (n + 1) * 128],
                            rhs=at,
                            start=first,
                            stop=last,
                        )
            for n in range(HQL):
                # alternate ACT/DVE for the PSUM evictions: halves the
                # serialized copy tail after each g-group's last matmul
                ot = opool.tile([128, SB], F32, name=f"ot{g}_{n}", tag="ot")
                if n % 2 == 0:
                    nc.scalar.copy(ot, ps_d[n])
                else:
                    nc.vector.tensor_copy(ot, ps_d[n])
                nc.sync.dma_start(
                    out=io["outT"][n * 128 : (n + 1) * 128, gsl], in_=ot
                )


def _stage_a_block(nc, tc, io, sb, ps, xpool, rpool, xts, emit_xt, weights,
                   cos2, sin2, ident, qt_sb, kt_sb, vs_sb):
    """Projections + RoPE for seq block sb."""
    ssl = slice(sb * SB, (sb + 1) * SB)
    ps_q = [
        ps.tile([128, SB], F32, name=f"psq{t}_{sb}", tag=f"psq{t}")
        for t in range(HQL)
    ]
    ps_k = ps.tile([128, SB], F32, name=f"psk_{sb}", tag="psk")
    ps_v = ps.tile([128, SB], F32, name=f"psv_{sb}", tag="psv")
    wq_sb, wk_sb, wv_sb = weights
    for c in range(NC_):
        if c % 2 == 0:
            if (sb, c // 2) not in xts:
                emit_xt(sb, c // 2, nc.scalar)
            xt2 = xts.pop((sb, c // 2))
        xt = xt2[:, c % 2, :]
        first, last = c == 0, c == NC_ - 1
        for t in range(HQL):
            nc.tensor.matmul(
                ps_q[t],
                lhsT=wq_sb[:, c, t * 128 : (t + 1) * 128],
                rhs=xt,
                start=first,
                stop=last,
            )
        nc.tensor.matmul(
            ps_k, lhsT=wk_sb[:, c, :], rhs=xt, start=first, stop=last
        )
        nc.tensor.matmul(
            ps_v, lhsT=wv_sb[:, c, :], rhs=xt, start=first, stop=last
        )

    # PSUM evictions. Only psq0/psq1 (B's score banks), psk (B's first out
    # bank) and psv must evict before B(qb) starts: ACT copies qc0/qck/qc1
    # up front (high priority so stray DMA triggers don't delay them), vts
    # on DVE. psq2/psq3 evictions + q2/q3 ropes are deferred into B(qb)'s
    # later heads - B's pss tiles rotate over psq0/psq1 only.
    qc_k = rpool.tile([128, SB], BF16, name=f"qck{sb}", tag="qck")
    qc_q = [
        rpool.tile([128, SB], BF16, name=f"qc{sb}_{t}", tag=f"qc{t}")
        for t in range(HQL)
    ]

    def rope_dve(qc, dst, idx):
        sw = rpool.tile([128, SB], BF16, name=f"sw{idx}", tag="sw")
        nc.vector.stream_shuffle(sw, qc, SWAP_MASK)
        t1 = rpool.tile([128, SB], BF16, name=f"t1{idx}", tag="t1")
        nc.vector.tensor_mul(t1, qc, cos2[:, ssl])
        t2 = rpool.tile([128, SB], BF16, name=f"t2{idx}", tag="t2")
        nc.vector.tensor_mul(t2, sw, sin2[:, ssl])
        nc.vector.tensor_add(dst, t1, t2)

    vts = rpool.tile([128, SB], BF16, name=f"vts{sb}", tag="vts")
    with tc.high_priority():
        if sb == 0:
            nc.scalar.copy(qc_k, ps_k)
            nc.scalar.copy(qc_q[0], ps_q[0])
        else:
            nc.scalar.copy(qc_q[0], ps_q[0])
            nc.scalar.copy(qc_k, ps_k)
        nc.scalar.copy(qc_q[1], ps_q[1])
        nc.vector.tensor_copy(vts, ps_v)
        if sb == 0:
            rope_dve(qc_k, kt_sb[sb], f"k{sb}")
            rope_dve(qc_q[0], qt_sb[sb][:, 0, :], f"q{sb}_0")
        else:
            rope_dve(qc_q[0], qt_sb[sb][:, 0, :], f"q{sb}_0")
            rope_dve(qc_k, kt_sb[sb], f"k{sb}")

    def defer_make(t, copy_needed):
        def emit():
            if copy_needed:
                nc.vector.tensor_copy(qc_q[t], ps_q[t])
            rope_dve(qc_q[t], qt_sb[sb][:, t, :], f"q{sb}_{t}")
        return emit

    deferred = {
        1: [defer_make(1, False)],
        2: [defer_make(2, True)],
        3: [defer_make(3, True)],
    }

    def transp():
        # V^T -> V (PE transpose per 128-col chunk); fills the PE's rope
        # wait at the B-block head, results needed first by pv(4*qb).
        for u in range(SB // 128):
            ps_vt = ps.tile([128, 128], BF16, name=f"psvt{sb}_{u}", tag="extra1")
            nc.tensor.transpose(ps_vt, vts[:, u * 128 : (u + 1) * 128], ident)
            nc.vector.tensor_copy(vs_sb[sb][:, u, :], ps_vt)

    return deferred, transp


def _stage_b_block(nc, tc, qb, ps, ppool, accp, spool, maskt, ones_mat,
                   qt_sb, kt_sb, vs_sb, ptd, attn_loc, attn_g,
                   deferred, transp):
    """Causal attention for q-block qb over k-chunks 0..4*(qb+1)."""
    nkc = 4 * (qb + 1)
    for h in range(HQL):
        for emit in deferred.pop(h, []):
            emit()
        pso = ps.tile(
            [128, SB], F32, name=f"pso{qb}_{h}", tag=("psk" if h % 2 == 0 else "psv")
        )
        psn = ps.tile([128, SB], F32, name=f"psn{qb}_{h}", tag="extra0")
        pts = {}
        pend = {}  # binary-counter pairwise tree accumulation on DVE

        def feed(t, _s=[0]):
            lvl = 0
            while lvl in pend:
                prev = pend.pop(lvl)
                _s[0] += 1
                nt = accp.tile(
                    [128, SB], BF16, name=f"acc{qb}_{h}_{_s[0]}", tag=f"acc{lvl}"
                )
                nc.vector.tensor_add(nt, prev, t)
                t = nt
                lvl += 1
            pend[lvl] = t

        def pv(kc):
            nc.tensor.matmul(
                pso,
                lhsT=vs_sb[kc // 4][:, kc % 4, :],
                rhs=pts.pop(kc),
                start=kc == 0,
                stop=kc == nkc - 1,
            )

        for kc in range(nkc):
            if h == 0 and kc == 2:
                transp()
            td = kc - 4 * qb
            pss = ps.tile(
                [128, SB], F32, name=f"pss{qb}_{h}_{kc}", tag=f"psq{kc % 2}"
            )
            ktc = kt_sb[kc // 4][:, (kc % 4) * 128 : (kc % 4 + 1) * 128]
            qtc = qt_sb[qb][:, h, :]
            if td < 0:
                nc.tensor.matmul(pss, lhsT=ktc, rhs=qtc, start=True, stop=True)
                pt = ppool.tile([128, SB], BF16, name=f"pt{qb}_{h}_{kc}", tag="pt")
                nc.scalar.activation(
                    pt, pss, mybir.ActivationFunctionType.Exp, scale=SCALE
                )
            else:
                lo = 128 * td
                nc.tensor.matmul(
                    pss[:, lo:], lhsT=ktc, rhs=qtc[:, lo:], start=True, stop=True
                )
                pt = ptd[td]
                nc.scalar.activation(
                    pt[:, lo:],
                    pss[:, lo:],
                    mybir.ActivationFunctionType.Exp,
                    scale=SCALE,
                )
                # triangular strip mask (in-place on the 128-wide strip)
                nc.vector.tensor_mul(
                    pt[:, lo : lo + 128], pt[:, lo : lo + 128], maskt
                )
            pts[kc] = pt
            feed(pt)
            if kc >= 2:
                pv(kc - 2)
        pv(nkc - 2)
        pv(nkc - 1)
        # combine leftover tree levels ascending -> root
        lvls = sorted(pend)
        root = pend[lvls[0]]
        for lv in lvls[1:]:
            nt = accp.tile([128, SB], BF16, name=f"accr{qb}_{h}_{lv}", tag="accr")
            nc.vector.tensor_add(nt, pend[lv], root)
            root = nt
        # ones_mat stationary => every partition of psn gets the column-sum:
        # the softmax denominator, already broadcast.
        nc.tensor.matmul(psn, lhsT=ones_mat, rhs=root, start=True, stop=True)
        rb = spool.tile([128, SB], F32, name=f"rb{qb}_{h}", tag="rb")
        nc.vector.reciprocal_approx_fast(rb, psn)
        ao = spool.tile([128, SB], BF16, name=f"ao{qb}_{h}", tag="ao", bufs=4)
        nc.vector.tensor_mul(ao, pso, rb)
        # ao DMA on gpsimd: precedes the gather trigger on the same queue
        nc.gpsimd.dma_start(out=attn_loc[qb][h, :, :], in_=ao)
    # one AllGather per q-block covering all 4 local heads
    nc.gpsimd.collective_compute(
        "AllGather",
        mybir.AluOpType.bypass,
        replica_groups=[list(range(NCORES))],
        ins=[attn_loc[qb].opt()],
        outs=[attn_g[qb].opt()],
    )


_NC_CACHE = None


def _get_nc():
    global _NC_CACHE
    if _NC_CACHE is None:
        _NC_CACHE = _build_nc()
    return _NC_CACHE


def _prep_in_maps(x, freqs_cos, freqs_sin, wq, wk, wv, wo):
    bf = ml_dtypes.bfloat16
    x = np.asarray(x, np.float32).reshape(S, D)
    xT = np.ascontiguousarray(x.T).astype(bf)
    cos = np.asarray(freqs_cos, np.float32)  # [S, HD/2]
    sin = np.asarray(freqs_sin, np.float32)
    cos2 = np.repeat(cos.T, 2, axis=0)  # [HD, S], rows 2j,2j+1 = cos[:, j]
    sin_t = sin.T
    sin2 = np.empty((HD, S), np.float32)
    sin2[0::2] = -sin_t
    sin2[1::2] = sin_t
    p = np.arange(128)[:, None]
    c = np.arange(128)[None, :]
    maskt = (p <= c).astype(bf)  # triangular strip mask, same for every td
    ident = np.eye(128, dtype=bf)
    wq = np.asarray(wq, np.float32)
    wk = np.asarray(wk, np.float32)
    wv = np.asarray(wv, np.float32)
    wo = np.asarray(wo, np.float32)
    in_maps = []
    for i in range(NCORES):
        in_maps.append(
            {
                "xT": xT,
                "cos2": cos2.astype(bf),
                "sin2": sin2.astype(bf),
                "maskt": maskt,
                "ident": ident,
                "wq": np.ascontiguousarray(
                    wq[:, i * HQL * HD : (i + 1) * HQL * HD]
                ).astype(bf),
                "wk": np.ascontiguousarray(wk[:, i * HD : (i + 1) * HD]).astype(bf),
                "wv": np.ascontiguousarray(wv[:, i * HD : (i + 1) * HD]).astype(bf),
                "wo": np.ascontiguousarray(
                    wo[:, i * HQL * HD : (i + 1) * HQL * HD]
                ).astype(bf),
            }
        )
    return in_maps


def _install_trace_shims():
    """The container's antenv lacks axon_hooks; replicate trn_boot's ctypes
    NTFF hook so run_bass_kernel_spmd(trace=True) works. Also stub out the
    fish-bucket artifact upload (no bucket access here)."""
    import sys
    import types
    import ctypes
    import contextlib

    if "antenv.axon_hooks" not in sys.modules:
        mod = types.ModuleType("antenv.axon_hooks")
        mod._hook = None

        def set_axon_ntff_profile_hook(h):
            mod._hook = h

        def get_axon_ntff_profile_hook():
            return mod._hook

        mod.set_axon_ntff_profile_hook = set_axon_ntff_profile_hook
        mod.get_axon_ntff_profile_hook = get_axon_ntff_profile_hook
        sys.modules["antenv.axon_hooks"] = mod

        so_path = "/opt/axon/libaxon_pjrt.so"
        lib = ctypes.CDLL(so_path)
        if hasattr(lib, "axon_start_nrt_profile"):
            lib.axon_start_nrt_profile.argtypes = [
                ctypes.POINTER(ctypes.c_int64),
                ctypes.c_size_t,
            ]
            lib.axon_start_nrt_profile.restype = ctypes.c_int64
            lib.axon_stop_nrt_profile.argtypes = [ctypes.c_char_p]
            lib.axon_stop_nrt_profile.restype = ctypes.c_int64

            @contextlib.contextmanager
            def _hook(output_dir, device_ids):
                import jax

                jax.devices()
                if device_ids:
                    ids = (ctypes.c_int64 * len(device_ids))(*device_ids)
                    rc = lib.axon_start_nrt_profile(ids, len(device_ids))
                else:
                    rc = lib.axon_start_nrt_profile(None, 0)
                if rc != 0:
                    raise RuntimeError(f"axon_start_nrt_profile rc={rc}")
                try:
                    yield
                finally:
                    n = lib.axon_stop_nrt_profile(str(output_dir).encode())
                    if n <= 0:
                        print(f"WARNING: axon_stop_nrt_profile rc={n}")

            set_axon_ntff_profile_hook(_hook)

    import concourse.bass_utils as bu

    bu.upload_artifacts = lambda tmpdir: "local://" + str(tmpdir)


def run(inputs, trace=False, **kw):
    nc = _get_nc()
    if trace:
        _install_trace_shims()
    in_maps = _prep_in_maps(**inputs)
    res = run_bass_kernel_spmd(nc, in_maps, list(range(NCORES)), trace=trace, **kw)
    out = np.concatenate(
        [res.results[i]["outT"].T for i in range(NCORES)], axis=1
    )
    return out.reshape(B, S, D).astype(np.float32), res


def kernel(x, freqs_cos, freqs_sin, wq, wk, wv, wo):
    out, _ = run(
        dict(
            x=x,
            freqs_cos=freqs_cos,
            freqs_sin=freqs_sin,
            wq=wq,
            wk=wk,
            wv=wv,
            wo=wo,
        )
    )
    return out
